# revision 50
# baseline (speedup 1.0000x reference)
"""Trainium2 Bass kernel for nn_BitwiseTasNetBlock (bf16 matmul version).

Model: 4 layers of [1x1 conv C->D, PReLU, BN, dilated depthwise conv K=3,
PReLU, BN, 1x1 conv D->C] with a residual around the whole stack.
B=8, C=128, D=512, T=8000. Training-mode BatchNorm -> stats over (batch, time).

Sharding: data-parallel over batch, one batch element per NeuronCore (8 cores).

v2 design (vs the fp32 baseline, 1.55ms -> ~0.63ms):
- All matmuls in bf16 (1 cyc/row on the PE vs 4 for fp32): conv1, the
  depthwise conv as diagonal-matmul taps, conv2. Activations are stored
  bf16 (written by the Act engine from fp32 PSUM), PSUM accumulation fp32.
  Measured rel err ~9.6e-3 (gate 2e-2), matching a numpy bf16 emulation.
- p1 and p2 share the A[g] buffers; internal super-tile drains split
  interior/tail so the next tile's DW left-halo reads p1 before PReLU2
  overwrites it.
- BN1 stats: per-channel sums ride the PReLU1 activations' accum_out;
  sums of squares via one scalar_tensor_tensor (p*p, fused row-sum) pass
  per group on the DVE. Cross-core reduction via two AllGather pairs in
  DW order {3,0}/{1,2} + local reduce (AllGather is ~2x lower latency
  than AllReduce; pair 0 resolves while conv1 of later groups still runs).
- BN1 affine split across engines to match queue idle slots:
  DVE (gather-reduce, var, 1/(var+eps)) -> Act (sqrt -> rstd, after the
  last PReLU1) -> Pool (tensor_tensor-only scale/bias products; Pool
  rejects TensorScalarPtr and any PSUM access).
- DW group order DWO=[3,0,1,2]; the first DW group raw-copies its PSUM
  to SBUF on the DVE (collective-independent) so the PE keeps its 2-tile
  PSUM runway through the BN1 gather window.
- BN2 stats: group 3 via P2 accum_out + DVE sumsq pass; groups 0-2 via
  bn_stats/bn_aggr (16 x 500-col chunks) pipelined behind the P2 drains.
  Two pair AllGathers; pair A hides under the later DW groups, and conv2
  accumulates pair-A groups first so its fills overlap pair B's gather.
- conv2 drains on Act (Identity + bias, same act table -> no reloads);
  the last layer is finalized by a fused DVE scalar_tensor_tensor
  (psum + bias) + x, so the residual needs no extra matmul or pass.
"""

import numpy as np
import ml_dtypes
from contextlib import ExitStack

import concourse.bass as bass
import concourse.bacc as bacc
import concourse.mybir as mybir
import concourse.tile as tile
from concourse.bass_utils import run_bass_kernel_spmd

F32 = mybir.dt.float32
BF = mybir.dt.bfloat16
AF = mybir.ActivationFunctionType
ALU = mybir.AluOpType

NCORES = 8
B, C, D, T, L, K = 8, 128, 512, 8000, 4, 3
G = D // 128          # 4 channel groups of 128 partitions
PAD = 8               # max dilation
W = T + 2 * PAD       # padded activation width
NTW = 512             # matmul free-dim tile
STW = 2048            # psum super-tile (4 banks of f32)
CH = 500              # bn_stats chunk (equal sizes -> exact bn_aggr)
NCH = T // CH         # 16 chunks
EPS = 1e-5
NTOT = float(NCORES * T)   # BN sample count over (batch, time)

ST_COLS = [(0, 2048), (2048, 4096), (4096, 6144), (6144, 8000)]
NST = len(ST_COLS)

VEC_TABLES = ["b1", "g1", "be1", "bd", "swI", "swL", "swR", "g2", "be2"]
VOFF = {t: j * (L * G) for j, t in enumerate(VEC_TABLES)}

# Group processing order for conv1 and the DW phase. The Act-stats group
# (3) goes first so its BN2 stats finish early; BN2 gathers in pairs
# {3,0} then {1,2}, so pair 0's collective hides under pair 1's DW work.
DWO = [3, 0, 1, 2]

LINEARIZE = False


def _build_program(alphas1, alphas2):
    nc = bacc.Bacc("TRN2", target_bir_lowering=False, debug=False, num_devices=NCORES)

    xin = nc.dram_tensor("xin", [128, T], F32, kind="ExternalInput")
    w1t = nc.dram_tensor("w1t", [128, L * D], BF, kind="ExternalInput")
    w2t = nc.dram_tensor("w2t", [128, L * D], BF, kind="ExternalInput")
    diag = nc.dram_tensor("diag", [128, L * G * K * 128], BF, kind="ExternalInput")
    vec = nc.dram_tensor("vec", [128, len(VEC_TABLES) * L * G], F32, kind="ExternalInput")
    b2d = nc.dram_tensor("b2d", [128, L], F32, kind="ExternalInput")
    yout = nc.dram_tensor("yout", [128, T], F32, kind="ExternalOutput")

    # collective bounce buffers: per layer, 4 per-group BN1 gathers (issued
    # as each group's stats land, so the latency hides under compute) and
    # 2 BN2 pair gathers (pair A hides under the later DW groups).
    # AllGather + local reduce is ~2x lower latency than AllReduce here.
    cin1 = [[nc.dram_tensor(f"cin1_{i}_{p}", [128, 4], F32) for p in range(2)]
            for i in range(L)]
    cout1 = [[nc.dram_tensor(f"cout1_{i}_{p}", [NCORES, 128, 4], F32,
                             addr_space="Shared") for p in range(2)]
             for i in range(L)]
    # BN2 ships as {first 3 DW groups} + {last group}: the 3-group gather
    # resolves while the last DW group still computes, and conv2 can prefill
    # 3 of its 4 accumulation blocks during the tiny final gather.
    BN2W = [6, 2]
    cin2 = [[nc.dram_tensor(f"cin2_{i}_{p}", [128, BN2W[p]], F32)
             for p in range(2)] for i in range(L)]
    cout2 = [[nc.dram_tensor(f"cout2_{i}_{p}", [NCORES, 128, BN2W[p]], F32,
                             addr_space="Shared") for p in range(2)]
             for i in range(L)]

    rgroups = [list(range(NCORES))]

    with tile.TileContext(nc, linearize=LINEARIZE) as tc, ExitStack() as ctx:
        # ---- persistent SBUF ----
        xs = nc.alloc_sbuf_tensor("xs", [128, T], F32)       # exact x (residual)
        xb = nc.alloc_sbuf_tensor("xb", [128, W], BF)        # bf16 x, halo-padded
        zraw = nc.alloc_sbuf_tensor("zraw", [128, T], BF)    # DW j0 raw psum
        hb = [nc.alloc_sbuf_tensor(f"hb{j}", [128, T], BF) for j in range(2)]
        A = [nc.alloc_sbuf_tensor(f"act{g}", [128, W], BF) for g in range(G)]
        w1s = nc.alloc_sbuf_tensor("w1s", [128, L * D], BF)
        w2s = nc.alloc_sbuf_tensor("w2s", [128, L * D], BF)
        vec_s = nc.alloc_sbuf_tensor("vecs", [128, len(VEC_TABLES) * L * G], F32)
        b2_s = nc.alloc_sbuf_tensor("b2s", [128, L], F32)

        psum = ctx.enter_context(tc.tile_pool(name="psum", bufs=2, space="PSUM"))
        diagp = ctx.enter_context(tc.tile_pool(name="diagp", bufs=2))
        small = ctx.enter_context(tc.tile_pool(name="small", bufs=2))
        # scratch outputs are never read back: single-buffered is free
        scrp = ctx.enter_context(tc.tile_pool(name="scrp", bufs=1))
        ysp = ctx.enter_context(tc.tile_pool(name="ysp", bufs=2))

        # ---- initial loads ----
        nc.sync.dma_start(out=w1s[:], in_=w1t[:])
        nc.sync.dma_start(out=w2s[:], in_=w2t[:])
        nc.sync.dma_start(out=vec_s[:], in_=vec[:])
        nc.sync.dma_start(out=b2_s[:], in_=b2d[:])
        nc.vector.memset(xb[:, 0:PAD], 0.0)
        nc.vector.memset(xb[:, PAD + T : W], 0.0)
        for a in A:
            nc.vector.memset(a[:, 0:PAD], 0.0)
            nc.vector.memset(a[:, PAD + T : W], 0.0)
        # x: fp32 into xs (kept for the final residual), bf16 convert into xb
        for c0 in range(0, T, 2000):
            nc.sync.dma_start(out=xs[:, c0 : c0 + 2000], in_=xin[:, c0 : c0 + 2000])
            nc.vector.tensor_copy(
                out=xb[:, PAD + c0 : PAD + c0 + 2000], in_=xs[:, c0 : c0 + 2000]
            )

        def vcol(tbl, i, g=None):
            off = VOFF[tbl] + i * G + (0 if g is None else g)
            return vec_s[:, off : off + (G if g is None else 1)]

        def h_ap(i, c0, c1):
            if i == 0:
                return xb[:, PAD + c0 : PAD + c1]
            return hb[(i - 1) % 2][:, c0:c1]

        for i in range(L):
            delta = 2 ** i
            a1v = float(alphas1[i])
            a2v = float(alphas2[i])
            last = i == L - 1

            dg = diagp.tile([128, G * K * 128], BF, tag="diag")
            nc.sync.dma_start(
                out=dg[:], in_=diag[:, i * G * K * 128 : (i + 1) * G * K * 128]
            )

            # per-layer small tiles
            acc1 = small.tile([128, G * NST], F32, tag="acc1")   # PReLU1 row sums
            q1 = small.tile([128, G * NST], F32, tag="q1")       # sumsq accums
            pk1 = small.tile([128, 2 * G], F32, tag="pk1")       # (sum, sumsq) / group
            red1 = small.tile([128, 2 * G], F32, tag="red1")     # AllReduced
            s1t = small.tile([128, G], F32, tag="s1t")
            t1t = small.tile([128, G], F32, tag="t1t")
            biasI = small.tile([128, G], F32, tag="biasI")
            biasL = small.tile([128, G], F32, tag="biasL")
            biasR = small.tile([128, G], F32, tag="biasR")
            mean1 = small.tile([128, G], F32, tag="mean1")
            ve1 = small.tile([128, G], F32, tag="ve1")
            sd1 = small.tile([128, G], F32, tag="sd1")
            bnst = small.tile([128, (G - 1) * NCH, 6], F32, tag="bnst")
            acc2 = small.tile([128, 8], F32, tag="acc2")         # g3 PReLU2 row sums
            q2 = small.tile([128, NST], F32, tag="q2")           # g3 Square row sums
            pk2 = small.tile([128, 2 * G], F32, tag="pk2")       # (mean, q) / group
            red2 = small.tile([128, 2 * G], F32, tag="red2")
            s2t = small.tile([128, G], F32, tag="s2t")
            t2bf = small.tile([128, G], BF, tag="t2bf")
            mean2 = small.tile([128, G], F32, tag="mean2")
            ve2 = small.tile([128, G], F32, tag="ve2")
            sd2 = small.tile([128, G], F32, tag="sd2")
            msq = small.tile([128, G], F32, tag="msq")
            w2sc = small.tile([128, D], BF, tag="w2sc")
            b2p = small.tile([128, 1], F32, tag="b2p")

            # ---- conv1 (C->D) + PReLU1 (+row-sum accum) + sumsq pass ----
            def conv1_group(j, g):
                lw = w1s[:, (i * G + g) * 128 : (i * G + g + 1) * 128]
                for st, (s0, s1c) in enumerate(ST_COLS):
                    ps = psum.tile([128, STW], F32, tag="big")
                    for n0 in range(s0, s1c, NTW):
                        n1 = min(n0 + NTW, s1c)
                        nc.tensor.matmul(
                            ps[:, n0 - s0 : n1 - s0], lw, h_ap(i, n0, n1),
                            start=True, stop=True,
                        )
                    col = j * NST + st
                    nc.scalar.activation(
                        out=A[g][:, PAD + s0 : PAD + s1c],
                        in_=ps[:, 0 : s1c - s0],
                        func=AF.Prelu,
                        bias=vcol("b1", i, g),
                        scale=1.0,
                        alpha=a1v,
                        accum_out=acc1[:, col : col + 1],
                    )
                    scr = scrp.tile([128, STW], BF, tag=f"sq{j % 2}")
                    nc.vector.scalar_tensor_tensor(
                        out=scr[:, 0 : s1c - s0],
                        in0=A[g][:, PAD + s0 : PAD + s1c],
                        scalar=1.0,
                        in1=A[g][:, PAD + s0 : PAD + s1c],
                        op0=ALU.mult,
                        op1=ALU.mult,
                        accum_out=q1[:, col : col + 1],
                    )
                # reduce the 4 per-ST accumulators into pk1 (j-indexed cols)
                nc.vector.tensor_reduce(
                    out=pk1[:, 2 * j : 2 * j + 1],
                    in_=acc1[:, j * NST : (j + 1) * NST],
                    axis=mybir.AxisListType.X, op=ALU.add,
                )
                nc.vector.tensor_reduce(
                    out=pk1[:, 2 * j + 1 : 2 * j + 2],
                    in_=q1[:, j * NST : (j + 1) * NST],
                    axis=mybir.AxisListType.X, op=ALU.add,
                )

            def trig1(p):
                nc.sync.dma_start(out=cin1[i][p][:], in_=pk1[:, 4 * p : 4 * p + 4])
                nc.gpsimd.collective_compute(
                    "AllGather", ALU.bypass, replica_groups=rgroups,
                    ins=[cin1[i][p][:]], outs=[cout1[i][p][:]],
                )

            # BN1 affine, split by engine so each piece lands in its queue's
            # natural idle slot: DVE (gather-reduce, var, 1/(var+eps)) ->
            # Act (sqrt -> rstd) -> Pool (scale/bias products).
            def aff1_dve(p):
                jc = slice(2 * p, 2 * p + 2)
                gat = small.tile([128, 4, NCORES], F32, tag=f"gat1{p}")
                nc.sync.dma_start(
                    out=gat[:], in_=cout1[i][p][:].rearrange("r p s -> p s r"))
                nc.vector.tensor_reduce(
                    out=red1[:, 4 * p : 4 * p + 4], in_=gat[:],
                    axis=mybir.AxisListType.X, op=ALU.add,
                )
                ev = red1[:, 4 * p : 4 * p + 4 : 2]
                od = red1[:, 4 * p + 1 : 4 * p + 4 : 2]
                nc.vector.tensor_scalar(mean1[:, jc], ev, 1.0 / NTOT, None, ALU.mult)
                nc.vector.tensor_scalar(
                    ve1[:, jc], od, 1.0 / NTOT, EPS, ALU.mult, ALU.add)
                nc.vector.tensor_mul(sd1[:, jc], mean1[:, jc], mean1[:, jc])
                nc.vector.tensor_sub(ve1[:, jc], ve1[:, jc], sd1[:, jc])
                nc.vector.reciprocal(out=sd1[:, jc], in_=ve1[:, jc])  # 1/(var+eps)

            def aff1_act(p):
                jc = slice(2 * p, 2 * p + 2)
                nc.scalar.activation(
                    out=ve1[:, jc], in_=sd1[:, jc], func=AF.Sqrt)     # rstd

            def aff1_pool(p):
                for jj in (2 * p, 2 * p + 1):
                    g = DWO[jj]
                    gc = slice(g, g + 1)
                    jjc = slice(jj, jj + 1)
                    nc.gpsimd.tensor_mul(s1t[:, gc], vcol("g1", i, g), ve1[:, jjc])
                    nc.gpsimd.tensor_mul(t1t[:, gc], mean1[:, jjc], s1t[:, gc])
                    nc.gpsimd.tensor_sub(t1t[:, gc], vcol("be1", i, g), t1t[:, gc])
                    for bt, tbl in ((biasI, "swI"), (biasL, "swL"), (biasR, "swR")):
                        nc.gpsimd.tensor_mul(bt[:, gc], t1t[:, gc], vcol(tbl, i, g))
                        nc.gpsimd.tensor_add(bt[:, gc], bt[:, gc], vcol("bd", i, g))

            conv1_group(0, DWO[0])
            conv1_group(1, DWO[1])
            trig1(0)
            conv1_group(2, DWO[2])
            aff1_dve(0)
            conv1_group(3, DWO[3])
            aff1_act(0)
            aff1_pool(0)
            trig1(1)

            # ---- depthwise dilated conv + PReLU2 (BN1 folded) + BN2 stats ----
            bn_emitted = [0] * G

            def emit_bn2(g, upto):
                # bn_stats chunks of CH cols, emitted once fully drained
                while bn_emitted[g] < NCH and (bn_emitted[g] + 1) * CH <= upto:
                    c = bn_emitted[g]
                    nc.vector.bn_stats(
                        out=bnst[:, g * NCH + c, :],
                        in_=A[g][:, PAD + c * CH : PAD + (c + 1) * CH],
                    )
                    bn_emitted[g] += 1

            for j, g in enumerate(DWO):
                if j == 2:
                    # pair 1's BN1 affine: the gather finished during the
                    # earlier DW groups, so no queue blocks here
                    aff1_dve(1)
                    aff1_act(1)
                    aff1_pool(1)
                actstats = j == 0   # first DW group's BN2 stats ride the Act
                n_inst = 0

                def dw_fill(st):
                    s0, s1c = ST_COLS[st]
                    ps = psum.tile([128, STW], F32, tag="big")
                    for k in range(K):
                        off = (k - 1) * delta
                        dw = dg[:, (g * K + k) * 128 : (g * K + k + 1) * 128]
                        for n0 in range(s0, s1c, NTW):
                            n1 = min(n0 + NTW, s1c)
                            nc.tensor.matmul(
                                ps[:, n0 - s0 : n1 - s0], dw,
                                A[g][:, PAD + n0 + off : PAD + n1 + off],
                                start=(k == 0), stop=(k == K - 1),
                            )
                    return ps

                def dw_drain(ps, st, part):
                    # p2 overwrites p1 in-place; the next super-tile's DW
                    # fill reads this tile's last `delta` cols as left halo,
                    # so internal tiles drain interior first, tail after
                    # that fill ("i"=interior, "t"=tail, "f"=full).
                    nonlocal n_inst
                    s0, s1c = ST_COLS[st]
                    w = s1c - s0
                    segs = []
                    if part in ("f", "i"):
                        e_hi = w if part == "f" else w - delta
                        if st == 0:
                            segs.append((0, delta, biasL))
                            segs.append((delta, e_hi, biasI))
                        elif st == NST - 1:
                            segs.append((0, w - delta, biasI))
                            segs.append((w - delta, w, biasR))
                        else:
                            segs.append((0, e_hi, biasI))
                    else:  # tail
                        segs.append((w - delta, w, biasI))
                    for e0, e1, bt in segs:
                        nc.scalar.activation(
                            out=A[g][:, PAD + s0 + e0 : PAD + s0 + e1],
                            in_=ps[:, e0:e1],
                            func=AF.Prelu,
                            bias=bt[:, g : g + 1],
                            scale=s1t[:, g : g + 1],
                            alpha=a2v,
                            accum_out=acc2[:, n_inst : n_inst + 1]
                            if actstats else None,
                        )
                        n_inst += 1
                    if part in ("f", "t"):
                        # tile fully drained: emit this tile's stats pass
                        if actstats:
                            sqs = scrp.tile([128, STW], BF, tag="sqa")
                            nc.scalar.activation(
                                out=sqs[:, 0:w],
                                in_=A[g][:, PAD + s0 : PAD + s1c],
                                func=AF.Square,
                                accum_out=q2[:, st : st + 1],
                            )
                        else:
                            emit_bn2(g, s1c)

                if j == 0:
                    # First DW group: raw-copy PSUM to SBUF on the DVE (no
                    # dependency on the BN1 collective), so the PE keeps its
                    # PSUM runway through the gather window; PReLU2 then
                    # reads the raw copy whenever the affine lands.
                    fl = [dw_fill(0), dw_fill(1)]
                    nc.vector.tensor_copy(out=zraw[:, 0:2048], in_=fl[0][:, 0:2048])
                    fl.append(dw_fill(2))
                    nc.vector.tensor_copy(out=zraw[:, 2048:4096], in_=fl[1][:, 0:2048])
                    fl.append(dw_fill(3))
                    nc.vector.tensor_copy(out=zraw[:, 4096:6144], in_=fl[2][:, 0:2048])
                    nc.vector.tensor_copy(out=zraw[:, 6144:8000], in_=fl[3][:, 0:1856])
                    for st, (s0, s1c) in enumerate(ST_COLS):
                        w = s1c - s0
                        segs = [(0, w, biasI)]
                        if st == 0:
                            segs = [(0, delta, biasL), (delta, w, biasI)]
                        elif st == NST - 1:
                            segs = [(0, w - delta, biasI), (w - delta, w, biasR)]
                        for e0, e1, bt in segs:
                            nc.scalar.activation(
                                out=A[g][:, PAD + s0 + e0 : PAD + s0 + e1],
                                in_=zraw[:, s0 + e0 : s0 + e1],
                                func=AF.Prelu,
                                bias=bt[:, g : g + 1],
                                scale=s1t[:, g : g + 1],
                                alpha=a2v,
                                accum_out=acc2[:, n_inst : n_inst + 1],
                            )
                            n_inst += 1
                        sqs = scrp.tile([128, STW], BF, tag="sqa")
                        nc.vector.scalar_tensor_tensor(
                            out=sqs[:, 0:w],
                            in0=A[g][:, PAD + s0 : PAD + s1c],
                            scalar=1.0,
                            in1=A[g][:, PAD + s0 : PAD + s1c],
                            op0=ALU.mult,
                            op1=ALU.mult,
                            accum_out=q2[:, st : st + 1],
                        )
                else:
                    ps0 = dw_fill(0)
                    ps1 = dw_fill(1)
                    dw_drain(ps0, 0, "f")
                    dw_drain(ps1, 1, "i")
                    ps2 = dw_fill(2)
                    dw_drain(ps1, 1, "t")
                    dw_drain(ps2, 2, "i")
                    ps3 = dw_fill(3)
                    dw_drain(ps2, 2, "t")
                    dw_drain(ps3, 3, "f")
                if actstats:
                    # (mean, E[p^2]) from the Act accumulators
                    nc.vector.tensor_reduce(
                        out=pk2[:, 2 * j : 2 * j + 1], in_=acc2[:, 0:n_inst],
                        axis=mybir.AxisListType.X, op=ALU.add)
                    nc.vector.tensor_reduce(
                        out=pk2[:, 2 * j + 1 : 2 * j + 2], in_=q2[:],
                        axis=mybir.AxisListType.X, op=ALU.add)
                    nc.vector.tensor_scalar(
                        pk2[:, 2 * j : 2 * j + 2], pk2[:, 2 * j : 2 * j + 2],
                        1.0 / T, None, ALU.mult)
                else:
                    # aggregate 16 chunks -> (mean, var); q = var + mean^2
                    nc.vector.bn_aggr(
                        out=pk2[:, 2 * j : 2 * j + 2],
                        in_=bnst[:, g * NCH : (g + 1) * NCH, :],
                    )
                    nc.vector.tensor_mul(
                        msq[:, j : j + 1], pk2[:, 2 * j : 2 * j + 1],
                        pk2[:, 2 * j : 2 * j + 1])
                    nc.vector.tensor_add(
                        pk2[:, 2 * j + 1 : 2 * j + 2], pk2[:, 2 * j + 1 : 2 * j + 2],
                        msq[:, j : j + 1])
                if j in (2, 3):
                    # ship (mean, q): groups {j0,j1,j2} early, {j3} at the end
                    p = j - 2
                    lo = 0 if p == 0 else 6
                    nc.sync.dma_start(
                        out=cin2[i][p][:], in_=pk2[:, lo : lo + BN2W[p]])
                    nc.gpsimd.collective_compute(
                        "AllGather", ALU.bypass, replica_groups=rgroups,
                        ins=[cin2[i][p][:]], outs=[cout2[i][p][:]],
                    )

            # ---- BN2 affine per pair + fold into conv2 weights ----
            def affine2_pair(p):
                lo = 0 if p == 0 else 6
                nj = BN2W[p] // 2            # groups in this shipment
                jlo = lo // 2
                jc = slice(jlo, jlo + nj)
                gat2 = small.tile([128, BN2W[p], NCORES], F32, tag=f"gat2{p}")
                nc.sync.dma_start(
                    out=gat2[:], in_=cout2[i][p][:].rearrange("r p s -> p s r"))
                nc.vector.tensor_reduce(
                    out=red2[:, lo : lo + BN2W[p]], in_=gat2[:],
                    axis=mybir.AxisListType.X, op=ALU.add)
                ev = red2[:, lo : lo + BN2W[p] : 2]
                od = red2[:, lo + 1 : lo + BN2W[p] : 2]
                nc.vector.tensor_scalar(mean2[:, jc], ev, 1.0 / NCORES, None, ALU.mult)
                nc.vector.tensor_scalar(
                    ve2[:, jc], od, 1.0 / NCORES, EPS, ALU.mult, ALU.add)
                nc.vector.tensor_mul(msq[:, jc], mean2[:, jc], mean2[:, jc])
                nc.vector.tensor_sub(ve2[:, jc], ve2[:, jc], msq[:, jc])
                # recip first, then Act sqrt -> rstd (the sqrt slots into the
                # Act idle window after the PReLU2 drains)
                nc.vector.reciprocal(out=sd2[:, jc], in_=ve2[:, jc])
                nc.scalar.activation(out=ve2[:, jc], in_=sd2[:, jc], func=AF.Sqrt)
                for jj in range(jlo, jlo + nj):
                    gx = DWO[jj]
                    jjc = slice(jj, jj + 1)
                    nc.vector.tensor_mul(s2t[:, jjc], vcol("g2", i, gx), ve2[:, jjc])
                    nc.vector.tensor_mul(msq[:, jjc], mean2[:, jjc], s2t[:, jjc])
                    nc.vector.tensor_sub(msq[:, jjc], vcol("be2", i, gx), msq[:, jjc])
                    nc.vector.tensor_copy(out=t2bf[:, jjc], in_=msq[:, jjc])
                    nc.vector.tensor_scalar(
                        w2sc[:, gx * 128 : (gx + 1) * 128],
                        w2s[:, (i * G + gx) * 128 : (i * G + gx + 1) * 128],
                        s2t[:, jjc], None, ALU.mult,
                    )

            affine2_pair(0)
            affine2_pair(1)
            mvp = psum.tile([128, STW], F32, tag="big")
            for jj in range(G):
                gx = DWO[jj]
                nc.tensor.matmul(
                    mvp[:, 0:1],
                    w2s[:, (i * G + gx) * 128 : (i * G + gx + 1) * 128],
                    t2bf[:, jj : jj + 1],
                    start=(jj == 0), stop=(jj == G - 1),
                )
            nc.vector.tensor_scalar(
                b2p[:], mvp[:, 0:1], b2_s[:, i : i + 1], None, ALU.add)

            # ---- conv2 (D->C); pair-0 groups accumulate first so their
            # fills overlap the pair-1 gather. The conv2 bias is NOT applied
            # at drain time (layers 0-2): it folds into the next layer's
            # PReLU1 bias (b1x = b1' + W1' @ b2p), keeping the drains off
            # the collective critical path.
            for st, (s0, s1c) in enumerate(ST_COLS):
                ps = psum.tile([128, STW], F32, tag="big")
                for jj in range(G):
                    gx = DWO[jj]
                    for n0 in range(s0, s1c, NTW):
                        n1 = min(n0 + NTW, s1c)
                        nc.tensor.matmul(
                            ps[:, n0 - s0 : n1 - s0],
                            w2sc[:, gx * 128 : (gx + 1) * 128],
                            A[gx][:, PAD + n0 : PAD + n1],
                            start=(jj == 0), stop=(jj == G - 1),
                        )
                if last:
                    ys = ysp.tile([128, STW], F32, tag="ys")
                    nc.vector.scalar_tensor_tensor(
                        out=ys[:, 0 : s1c - s0],
                        in0=ps[:, 0 : s1c - s0],
                        scalar=b2p[:, 0:1],
                        in1=xs[:, s0:s1c],
                        op0=ALU.add, op1=ALU.add,
                    )
                    nc.sync.dma_start(out=yout[:, s0:s1c], in_=ys[:, 0 : s1c - s0])
                else:
                    nc.scalar.activation(
                        out=hb[i % 2][:, s0:s1c], in_=ps[:, 0 : s1c - s0],
                        func=AF.Identity, bias=b2p[:, 0:1], scale=1.0,
                    )


    nc.finalize()
    return nc


_CACHE = {}


def _get_program(a1, a2):
    key = (tuple(np.asarray(a1, dtype=np.float64)), tuple(np.asarray(a2, dtype=np.float64)))
    if key not in _CACHE:
        _CACHE[key] = _build_program(np.asarray(a1), np.asarray(a2))
    return _CACHE[key]


def _pack_params(w1, b1, g1, be1, wd, bd, g2, be2, w2, b2):
    w1 = np.asarray(w1, np.float32)
    w2 = np.asarray(w2, np.float32)
    wd = np.asarray(wd, np.float32)

    w1t = np.concatenate([w1[i].T for i in range(L)], axis=1)  # [C, L*D]
    w2t = np.concatenate(
        [w2[i].T[g * 128 : (g + 1) * 128] for i in range(L) for g in range(G)],
        axis=1,
    )
    dblocks = []
    for i in range(L):
        for g in range(G):
            for k in range(K):
                dblocks.append(np.diag(wd[i, g * 128 : (g + 1) * 128, k]))
    diag = np.concatenate(dblocks, axis=1)

    def pack16(tbl):
        out = np.empty((128, L * G), np.float32)
        for i in range(L):
            for g in range(G):
                out[:, i * G + g] = tbl[i, g * 128 : (g + 1) * 128]
        return out

    sw = wd.sum(axis=2)
    swL = wd[:, :, 1] + wd[:, :, 2]
    swR = wd[:, :, 0] + wd[:, :, 1]
    tables = {
        "b1": pack16(np.asarray(b1, np.float32)),
        "g1": pack16(np.asarray(g1, np.float32)),
        "be1": pack16(np.asarray(be1, np.float32)),
        "bd": pack16(np.asarray(bd, np.float32)),
        "swI": pack16(sw),
        "swL": pack16(swL),
        "swR": pack16(swR),
        "g2": pack16(np.asarray(g2, np.float32)),
        "be2": pack16(np.asarray(be2, np.float32)),
    }
    vec = np.concatenate([tables[t] for t in VEC_TABLES], axis=1)
    b2dv = np.asarray(b2, np.float32).T.copy()
    bfc = lambda a: np.ascontiguousarray(a).astype(ml_dtypes.bfloat16)
    return {
        "w1t": bfc(w1t),
        "w2t": bfc(w2t),
        "diag": bfc(diag),
        "vec": np.ascontiguousarray(vec),
        "b2d": b2dv,
    }


def kernel(x, w1, b1, a1, g1, be1, wd, bd, a2, g2, be2, w2, b2, _trace=False):
    x = np.asarray(x, np.float32)
    nc = _get_program(a1, a2)
    params = _pack_params(w1, b1, g1, be1, wd, bd, g2, be2, w2, b2)
    in_maps = [{"xin": np.ascontiguousarray(x[c]), **params} for c in range(NCORES)]
    res = run_bass_kernel_spmd(nc, in_maps, list(range(NCORES)), trace=_trace)
    out = np.stack([res.results[c]["yout"] for c in range(NCORES)], axis=0)
    kernel._last_result = res
    return out.astype(np.float32)


# revision 51
# speedup vs baseline: 1.0676x; 1.0676x over previous
"""Trainium2 Bass kernel for nn_BitwiseTasNetBlock (bf16 matmul version).

Model: 4 layers of [1x1 conv C->D, PReLU, BN, dilated depthwise conv K=3,
PReLU, BN, 1x1 conv D->C] with a residual around the whole stack.
B=8, C=128, D=512, T=8000. Training-mode BatchNorm -> stats over (batch, time).

Sharding: data-parallel over batch, one batch element per NeuronCore (8 cores).

v2 design (vs the fp32 baseline, 1.55ms -> ~0.63ms):
- All matmuls in bf16 (1 cyc/row on the PE vs 4 for fp32): conv1, the
  depthwise conv as diagonal-matmul taps, conv2. Activations are stored
  bf16 (written by the Act engine from fp32 PSUM), PSUM accumulation fp32.
  Measured rel err ~9.6e-3 (gate 2e-2), matching a numpy bf16 emulation.
- p1 and p2 share the A[g] buffers; internal super-tile drains split
  interior/tail so the next tile's DW left-halo reads p1 before PReLU2
  overwrites it.
- BN1 stats: per-channel sums ride the PReLU1 activations' accum_out;
  sums of squares via one scalar_tensor_tensor (p*p, fused row-sum) pass
  per group on the DVE. Cross-core reduction via two AllGather pairs in
  DW order {3,0}/{1,2} + local reduce (AllGather is ~2x lower latency
  than AllReduce; pair 0 resolves while conv1 of later groups still runs).
- BN1 affine split across engines to match queue idle slots:
  DVE (gather-reduce, var, 1/(var+eps)) -> Act (sqrt -> rstd, after the
  last PReLU1) -> Pool (tensor_tensor-only scale/bias products; Pool
  rejects TensorScalarPtr and any PSUM access).
- DW group order DWO=[3,0,1,2]; the first DW group raw-copies its PSUM
  to SBUF on the DVE (collective-independent) so the PE keeps its 2-tile
  PSUM runway through the BN1 gather window.
- BN2 stats: group 3 via P2 accum_out + DVE sumsq pass; groups 0-2 via
  bn_stats/bn_aggr (16 x 500-col chunks) pipelined behind the P2 drains.
  Two pair AllGathers; pair A hides under the later DW groups, and conv2
  accumulates pair-A groups first so its fills overlap pair B's gather.
- conv2 drains on Act (Identity + bias, same act table -> no reloads);
  the last layer is finalized by a fused DVE scalar_tensor_tensor
  (psum + bias) + x, so the residual needs no extra matmul or pass.
"""

import numpy as np
import ml_dtypes
from contextlib import ExitStack

import concourse.bass as bass
import concourse.bacc as bacc
import concourse.mybir as mybir
import concourse.tile as tile
from concourse.bass_utils import run_bass_kernel_spmd

F32 = mybir.dt.float32
BF = mybir.dt.bfloat16
AF = mybir.ActivationFunctionType
ALU = mybir.AluOpType

NCORES = 8
B, C, D, T, L, K = 8, 128, 512, 8000, 4, 3
G = D // 128          # 4 channel groups of 128 partitions
PAD = 8               # max dilation
W = T + 2 * PAD       # padded activation width
NTW = 512             # matmul free-dim tile
STW = 2048            # psum super-tile (4 banks of f32)
CH = 500              # bn_stats chunk (equal sizes -> exact bn_aggr)
NCH = T // CH         # 16 chunks
EPS = 1e-5
NTOT = float(NCORES * T)   # BN sample count over (batch, time)

ST_COLS = [(0, 2048), (2048, 4096), (4096, 6144), (6144, 8000)]
NST = len(ST_COLS)

VEC_TABLES = ["b1", "g1", "be1", "bd", "swI", "swL", "swR", "g2", "be2"]
VOFF = {t: j * (L * G) for j, t in enumerate(VEC_TABLES)}

# Group processing order for conv1 and the DW phase. The Act-stats group
# (3) goes first so its BN2 stats finish early; BN2 gathers in pairs
# {3,0} then {1,2}, so pair 0's collective hides under pair 1's DW work.
DWO = [3, 0, 1, 2]

LINEARIZE = False


def _build_program(alphas1, alphas2):
    nc = bacc.Bacc("TRN2", target_bir_lowering=False, debug=False, num_devices=NCORES)

    xin = nc.dram_tensor("xin", [128, T], F32, kind="ExternalInput")
    w1t = nc.dram_tensor("w1t", [128, L * D], BF, kind="ExternalInput")
    w2t = nc.dram_tensor("w2t", [128, L * D], BF, kind="ExternalInput")
    diag = nc.dram_tensor("diag", [128, L * G * K * 128], BF, kind="ExternalInput")
    vec = nc.dram_tensor("vec", [128, len(VEC_TABLES) * L * G], F32, kind="ExternalInput")
    b2d = nc.dram_tensor("b2d", [128, L], F32, kind="ExternalInput")
    yout = nc.dram_tensor("yout", [128, T], F32, kind="ExternalOutput")

    # collective bounce buffers: per layer, 4 per-group BN1 gathers (issued
    # as each group's stats land, so the latency hides under compute) and
    # 2 BN2 pair gathers (pair A hides under the later DW groups).
    # AllGather + local reduce is ~2x lower latency than AllReduce here.
    cin1 = [[nc.dram_tensor(f"cin1_{i}_{p}", [128, 4], F32) for p in range(2)]
            for i in range(L)]
    cout1 = [[nc.dram_tensor(f"cout1_{i}_{p}", [NCORES, 128, 4], F32,
                             addr_space="Shared") for p in range(2)]
             for i in range(L)]
    # BN2 ships as {first 3 DW groups} + {last group}: the 3-group gather
    # resolves while the last DW group still computes, and conv2 can prefill
    # 3 of its 4 accumulation blocks during the tiny final gather.
    BN2W = [4, 4]
    cin2 = [[nc.dram_tensor(f"cin2_{i}_{p}", [128, BN2W[p]], F32)
             for p in range(2)] for i in range(L)]
    cout2 = [[nc.dram_tensor(f"cout2_{i}_{p}", [NCORES, 128, BN2W[p]], F32,
                             addr_space="Shared") for p in range(2)]
             for i in range(L)]

    rgroups = [list(range(NCORES))]

    with tile.TileContext(nc, linearize=LINEARIZE) as tc, ExitStack() as ctx:
        # ---- persistent SBUF ----
        xs = nc.alloc_sbuf_tensor("xs", [128, T], F32)       # exact x (residual)
        xb = nc.alloc_sbuf_tensor("xb", [128, W], BF)        # bf16 x, halo-padded
        zraw = nc.alloc_sbuf_tensor("zraw", [128, T], BF)    # DW j0 raw psum
        hb = [nc.alloc_sbuf_tensor(f"hb{j}", [128, T], BF) for j in range(2)]
        A = [nc.alloc_sbuf_tensor(f"act{g}", [128, W], BF) for g in range(G)]
        w1s = nc.alloc_sbuf_tensor("w1s", [128, L * D], BF)
        w2s = nc.alloc_sbuf_tensor("w2s", [128, L * D], BF)
        vec_s = nc.alloc_sbuf_tensor("vecs", [128, len(VEC_TABLES) * L * G], F32)
        b2_s = nc.alloc_sbuf_tensor("b2s", [128, L], F32)

        psum = ctx.enter_context(tc.tile_pool(name="psum", bufs=2, space="PSUM"))
        diagp = ctx.enter_context(tc.tile_pool(name="diagp", bufs=2))
        small = ctx.enter_context(tc.tile_pool(name="small", bufs=2))
        # scratch outputs are never read back: single-buffered is free
        scrp = ctx.enter_context(tc.tile_pool(name="scrp", bufs=1))
        ysp = ctx.enter_context(tc.tile_pool(name="ysp", bufs=2))

        # ---- initial loads ----
        nc.sync.dma_start(out=w1s[:], in_=w1t[:])
        nc.sync.dma_start(out=w2s[:], in_=w2t[:])
        nc.sync.dma_start(out=vec_s[:], in_=vec[:])
        nc.sync.dma_start(out=b2_s[:], in_=b2d[:])
        nc.vector.memset(xb[:, 0:PAD], 0.0)
        nc.vector.memset(xb[:, PAD + T : W], 0.0)
        for a in A:
            nc.vector.memset(a[:, 0:PAD], 0.0)
            nc.vector.memset(a[:, PAD + T : W], 0.0)
        # x: fp32 into xs (kept for the final residual), bf16 convert into xb
        for c0 in range(0, T, 2000):
            nc.sync.dma_start(out=xs[:, c0 : c0 + 2000], in_=xin[:, c0 : c0 + 2000])
            nc.vector.tensor_copy(
                out=xb[:, PAD + c0 : PAD + c0 + 2000], in_=xs[:, c0 : c0 + 2000]
            )

        def vcol(tbl, i, g=None):
            off = VOFF[tbl] + i * G + (0 if g is None else g)
            return vec_s[:, off : off + (G if g is None else 1)]

        def h_ap(i, c0, c1):
            if i == 0:
                return xb[:, PAD + c0 : PAD + c1]
            return hb[(i - 1) % 2][:, c0:c1]

        for i in range(L):
            delta = 2 ** i
            a1v = float(alphas1[i])
            a2v = float(alphas2[i])
            last = i == L - 1

            dg = diagp.tile([128, G * K * 128], BF, tag="diag")
            nc.sync.dma_start(
                out=dg[:], in_=diag[:, i * G * K * 128 : (i + 1) * G * K * 128]
            )

            # per-layer small tiles
            acc1 = small.tile([128, G * NST], F32, tag="acc1")   # PReLU1 row sums
            q1 = small.tile([128, G * NST], F32, tag="q1")       # sumsq accums
            pk1 = small.tile([128, 2 * G], F32, tag="pk1")       # (sum, sumsq) / group
            red1 = small.tile([128, 2 * G], F32, tag="red1")     # AllReduced
            s1t = small.tile([128, G], F32, tag="s1t")
            t1t = small.tile([128, G], F32, tag="t1t")
            biasI = small.tile([128, G], F32, tag="biasI")
            biasL = small.tile([128, G], F32, tag="biasL")
            biasR = small.tile([128, G], F32, tag="biasR")
            mean1 = small.tile([128, G], F32, tag="mean1")
            ve1 = small.tile([128, G], F32, tag="ve1")
            sd1 = small.tile([128, G], F32, tag="sd1")
            bnst = small.tile([128, (G - 1) * NCH, 6], F32, tag="bnst")
            acc2 = small.tile([128, 8], F32, tag="acc2")         # g3 PReLU2 row sums
            q2 = small.tile([128, NST], F32, tag="q2")           # g3 Square row sums
            pk2 = small.tile([128, 2 * G], F32, tag="pk2")       # (mean, q) / group
            red2 = small.tile([128, 2 * G], F32, tag="red2")
            s2t = small.tile([128, G], F32, tag="s2t")
            t2bf = small.tile([128, G], BF, tag="t2bf")
            mean2 = small.tile([128, G], F32, tag="mean2")
            ve2 = small.tile([128, G], F32, tag="ve2")
            sd2 = small.tile([128, G], F32, tag="sd2")
            msq = small.tile([128, G], F32, tag="msq")
            w2sc = small.tile([128, D], BF, tag="w2sc")
            b2p = small.tile([128, 1], F32, tag="b2p")

            # ---- conv1 (C->D) + PReLU1 (+row-sum accum) + sumsq pass ----
            def conv1_group(j, g):
                lw = w1s[:, (i * G + g) * 128 : (i * G + g + 1) * 128]
                for st, (s0, s1c) in enumerate(ST_COLS):
                    ps = psum.tile([128, STW], F32, tag="big")
                    for n0 in range(s0, s1c, NTW):
                        n1 = min(n0 + NTW, s1c)
                        nc.tensor.matmul(
                            ps[:, n0 - s0 : n1 - s0], lw, h_ap(i, n0, n1),
                            start=True, stop=True,
                        )
                    col = j * NST + st
                    nc.scalar.activation(
                        out=A[g][:, PAD + s0 : PAD + s1c],
                        in_=ps[:, 0 : s1c - s0],
                        func=AF.Prelu,
                        bias=vcol("b1", i, g),
                        scale=1.0,
                        alpha=a1v,
                        accum_out=acc1[:, col : col + 1],
                    )
                    scr = scrp.tile([128, STW], BF, tag=f"sq{j % 2}")
                    nc.vector.scalar_tensor_tensor(
                        out=scr[:, 0 : s1c - s0],
                        in0=A[g][:, PAD + s0 : PAD + s1c],
                        scalar=1.0,
                        in1=A[g][:, PAD + s0 : PAD + s1c],
                        op0=ALU.mult,
                        op1=ALU.mult,
                        accum_out=q1[:, col : col + 1],
                    )
                # reduce the 4 per-ST accumulators into pk1 (j-indexed cols)
                nc.vector.tensor_reduce(
                    out=pk1[:, 2 * j : 2 * j + 1],
                    in_=acc1[:, j * NST : (j + 1) * NST],
                    axis=mybir.AxisListType.X, op=ALU.add,
                )
                nc.vector.tensor_reduce(
                    out=pk1[:, 2 * j + 1 : 2 * j + 2],
                    in_=q1[:, j * NST : (j + 1) * NST],
                    axis=mybir.AxisListType.X, op=ALU.add,
                )

            def trig1(p):
                nc.sync.dma_start(out=cin1[i][p][:], in_=pk1[:, 4 * p : 4 * p + 4])
                nc.gpsimd.collective_compute(
                    "AllGather", ALU.bypass, replica_groups=rgroups,
                    ins=[cin1[i][p][:]], outs=[cout1[i][p][:]],
                )

            # BN1 affine, split by engine so each piece lands in its queue's
            # natural idle slot: DVE (gather-reduce, var, 1/(var+eps)) ->
            # Act (sqrt -> rstd) -> Pool (scale/bias products).
            def aff1_dve(p):
                jc = slice(2 * p, 2 * p + 2)
                gat = small.tile([128, 4, NCORES], F32, tag=f"gat1{p}")
                nc.sync.dma_start(
                    out=gat[:], in_=cout1[i][p][:].rearrange("r p s -> p s r"))
                nc.vector.tensor_reduce(
                    out=red1[:, 4 * p : 4 * p + 4], in_=gat[:],
                    axis=mybir.AxisListType.X, op=ALU.add,
                )
                ev = red1[:, 4 * p : 4 * p + 4 : 2]
                od = red1[:, 4 * p + 1 : 4 * p + 4 : 2]
                nc.vector.tensor_scalar(mean1[:, jc], ev, 1.0 / NTOT, None, ALU.mult)
                nc.vector.tensor_scalar(
                    ve1[:, jc], od, 1.0 / NTOT, EPS, ALU.mult, ALU.add)
                nc.vector.tensor_mul(sd1[:, jc], mean1[:, jc], mean1[:, jc])
                nc.vector.tensor_sub(ve1[:, jc], ve1[:, jc], sd1[:, jc])
                nc.vector.reciprocal(out=sd1[:, jc], in_=ve1[:, jc])  # 1/(var+eps)

            def aff1_act(p):
                jc = slice(2 * p, 2 * p + 2)
                nc.scalar.activation(
                    out=ve1[:, jc], in_=sd1[:, jc], func=AF.Sqrt)     # rstd

            def aff1_pool(p):
                for jj in (2 * p, 2 * p + 1):
                    g = DWO[jj]
                    gc = slice(g, g + 1)
                    jjc = slice(jj, jj + 1)
                    nc.gpsimd.tensor_mul(s1t[:, gc], vcol("g1", i, g), ve1[:, jjc])
                    nc.gpsimd.tensor_mul(t1t[:, gc], mean1[:, jjc], s1t[:, gc])
                    nc.gpsimd.tensor_sub(t1t[:, gc], vcol("be1", i, g), t1t[:, gc])
                    for bt, tbl in ((biasI, "swI"), (biasL, "swL"), (biasR, "swR")):
                        nc.gpsimd.tensor_mul(bt[:, gc], t1t[:, gc], vcol(tbl, i, g))
                        nc.gpsimd.tensor_add(bt[:, gc], bt[:, gc], vcol("bd", i, g))

            conv1_group(0, DWO[0])
            conv1_group(1, DWO[1])
            trig1(0)
            conv1_group(2, DWO[2])
            aff1_dve(0)
            conv1_group(3, DWO[3])
            aff1_act(0)
            aff1_pool(0)
            trig1(1)

            # ---- depthwise dilated conv + PReLU2 (BN1 folded) + BN2 stats ----
            bn_emitted = [0] * G

            def emit_bn2(g, upto):
                # bn_stats chunks of CH cols, emitted once fully drained
                while bn_emitted[g] < NCH and (bn_emitted[g] + 1) * CH <= upto:
                    c = bn_emitted[g]
                    nc.vector.bn_stats(
                        out=bnst[:, g * NCH + c, :],
                        in_=A[g][:, PAD + c * CH : PAD + (c + 1) * CH],
                    )
                    bn_emitted[g] += 1

            for j, g in enumerate(DWO):
                if j == 2:
                    # pair 1's BN1 affine: the gather finished during the
                    # earlier DW groups, so no queue blocks here
                    aff1_dve(1)
                    aff1_act(1)
                    aff1_pool(1)
                actstats = j == 0   # first DW group's BN2 stats ride the Act
                n_inst = 0

                def dw_fill(st):
                    s0, s1c = ST_COLS[st]
                    ps = psum.tile([128, STW], F32, tag="big")
                    for k in range(K):
                        off = (k - 1) * delta
                        dw = dg[:, (g * K + k) * 128 : (g * K + k + 1) * 128]
                        for n0 in range(s0, s1c, NTW):
                            n1 = min(n0 + NTW, s1c)
                            nc.tensor.matmul(
                                ps[:, n0 - s0 : n1 - s0], dw,
                                A[g][:, PAD + n0 + off : PAD + n1 + off],
                                start=(k == 0), stop=(k == K - 1),
                            )
                    return ps

                def dw_drain(ps, st, part):
                    # p2 overwrites p1 in-place; the next super-tile's DW
                    # fill reads this tile's last `delta` cols as left halo,
                    # so internal tiles drain interior first, tail after
                    # that fill ("i"=interior, "t"=tail, "f"=full).
                    nonlocal n_inst
                    s0, s1c = ST_COLS[st]
                    w = s1c - s0
                    segs = []
                    if part in ("f", "i"):
                        e_hi = w if part == "f" else w - delta
                        if st == 0:
                            segs.append((0, delta, biasL))
                            segs.append((delta, e_hi, biasI))
                        elif st == NST - 1:
                            segs.append((0, w - delta, biasI))
                            segs.append((w - delta, w, biasR))
                        else:
                            segs.append((0, e_hi, biasI))
                    else:  # tail
                        segs.append((w - delta, w, biasI))
                    for e0, e1, bt in segs:
                        nc.scalar.activation(
                            out=A[g][:, PAD + s0 + e0 : PAD + s0 + e1],
                            in_=ps[:, e0:e1],
                            func=AF.Prelu,
                            bias=bt[:, g : g + 1],
                            scale=s1t[:, g : g + 1],
                            alpha=a2v,
                            accum_out=acc2[:, n_inst : n_inst + 1]
                            if actstats else None,
                        )
                        n_inst += 1
                    if part in ("f", "t"):
                        # tile fully drained: emit this tile's stats pass
                        if actstats:
                            sqs = scrp.tile([128, STW], BF, tag="sqa")
                            nc.scalar.activation(
                                out=sqs[:, 0:w],
                                in_=A[g][:, PAD + s0 : PAD + s1c],
                                func=AF.Square,
                                accum_out=q2[:, st : st + 1],
                            )
                        else:
                            emit_bn2(g, s1c)

                if j == 0:
                    # First DW group: raw-copy PSUM to SBUF on the DVE (no
                    # dependency on the BN1 collective), so the PE keeps its
                    # PSUM runway through the gather window; PReLU2 then
                    # reads the raw copy whenever the affine lands.
                    fl = [dw_fill(0), dw_fill(1)]
                    nc.vector.tensor_copy(out=zraw[:, 0:2048], in_=fl[0][:, 0:2048])
                    fl.append(dw_fill(2))
                    nc.vector.tensor_copy(out=zraw[:, 2048:4096], in_=fl[1][:, 0:2048])
                    fl.append(dw_fill(3))
                    nc.vector.tensor_copy(out=zraw[:, 4096:6144], in_=fl[2][:, 0:2048])
                    nc.vector.tensor_copy(out=zraw[:, 6144:8000], in_=fl[3][:, 0:1856])
                    for st, (s0, s1c) in enumerate(ST_COLS):
                        w = s1c - s0
                        segs = [(0, w, biasI)]
                        if st == 0:
                            segs = [(0, delta, biasL), (delta, w, biasI)]
                        elif st == NST - 1:
                            segs = [(0, w - delta, biasI), (w - delta, w, biasR)]
                        for e0, e1, bt in segs:
                            nc.scalar.activation(
                                out=A[g][:, PAD + s0 + e0 : PAD + s0 + e1],
                                in_=zraw[:, s0 + e0 : s0 + e1],
                                func=AF.Prelu,
                                bias=bt[:, g : g + 1],
                                scale=s1t[:, g : g + 1],
                                alpha=a2v,
                                accum_out=acc2[:, n_inst : n_inst + 1],
                            )
                            n_inst += 1
                        sqs = scrp.tile([128, STW], BF, tag="sqa")
                        nc.vector.scalar_tensor_tensor(
                            out=sqs[:, 0:w],
                            in0=A[g][:, PAD + s0 : PAD + s1c],
                            scalar=1.0,
                            in1=A[g][:, PAD + s0 : PAD + s1c],
                            op0=ALU.mult,
                            op1=ALU.mult,
                            accum_out=q2[:, st : st + 1],
                        )
                else:
                    ps0 = dw_fill(0)
                    ps1 = dw_fill(1)
                    dw_drain(ps0, 0, "f")
                    dw_drain(ps1, 1, "i")
                    ps2 = dw_fill(2)
                    dw_drain(ps1, 1, "t")
                    dw_drain(ps2, 2, "i")
                    ps3 = dw_fill(3)
                    dw_drain(ps2, 2, "t")
                    dw_drain(ps3, 3, "f")
                if actstats:
                    # (mean, E[p^2]) from the Act accumulators
                    nc.vector.tensor_reduce(
                        out=pk2[:, 2 * j : 2 * j + 1], in_=acc2[:, 0:n_inst],
                        axis=mybir.AxisListType.X, op=ALU.add)
                    nc.vector.tensor_reduce(
                        out=pk2[:, 2 * j + 1 : 2 * j + 2], in_=q2[:],
                        axis=mybir.AxisListType.X, op=ALU.add)
                    nc.vector.tensor_scalar(
                        pk2[:, 2 * j : 2 * j + 2], pk2[:, 2 * j : 2 * j + 2],
                        1.0 / T, None, ALU.mult)
                else:
                    # aggregate 16 chunks -> (mean, var); q = var + mean^2
                    nc.vector.bn_aggr(
                        out=pk2[:, 2 * j : 2 * j + 2],
                        in_=bnst[:, g * NCH : (g + 1) * NCH, :],
                    )
                    nc.vector.tensor_mul(
                        msq[:, j : j + 1], pk2[:, 2 * j : 2 * j + 1],
                        pk2[:, 2 * j : 2 * j + 1])
                    nc.vector.tensor_add(
                        pk2[:, 2 * j + 1 : 2 * j + 2], pk2[:, 2 * j + 1 : 2 * j + 2],
                        msq[:, j : j + 1])
                if j in (1, 3):
                    # ship (mean, q): pair {j0,j1} early, {j2,j3} at the end
                    p = j // 2
                    lo = 0 if p == 0 else 4
                    nc.sync.dma_start(
                        out=cin2[i][p][:], in_=pk2[:, lo : lo + BN2W[p]])
                    nc.gpsimd.collective_compute(
                        "AllGather", ALU.bypass, replica_groups=rgroups,
                        ins=[cin2[i][p][:]], outs=[cout2[i][p][:]],
                    )

            # ---- BN2 affine per pair + fold into conv2 weights ----
            def affine2_pair(p):
                lo = 0 if p == 0 else 4
                nj = BN2W[p] // 2            # groups in this shipment
                jlo = lo // 2
                jc = slice(jlo, jlo + nj)
                gat2 = small.tile([128, BN2W[p], NCORES], F32, tag=f"gat2{p}")
                nc.sync.dma_start(
                    out=gat2[:], in_=cout2[i][p][:].rearrange("r p s -> p s r"))
                nc.vector.tensor_reduce(
                    out=red2[:, lo : lo + BN2W[p]], in_=gat2[:],
                    axis=mybir.AxisListType.X, op=ALU.add)
                ev = red2[:, lo : lo + BN2W[p] : 2]
                od = red2[:, lo + 1 : lo + BN2W[p] : 2]
                nc.vector.tensor_scalar(mean2[:, jc], ev, 1.0 / NCORES, None, ALU.mult)
                nc.vector.tensor_scalar(
                    ve2[:, jc], od, 1.0 / NCORES, EPS, ALU.mult, ALU.add)
                nc.vector.tensor_mul(msq[:, jc], mean2[:, jc], mean2[:, jc])
                nc.vector.tensor_sub(ve2[:, jc], ve2[:, jc], msq[:, jc])
                # recip first, then Act sqrt -> rstd (the sqrt slots into the
                # Act idle window after the PReLU2 drains)
                nc.vector.reciprocal(out=sd2[:, jc], in_=ve2[:, jc])
                nc.scalar.activation(out=ve2[:, jc], in_=sd2[:, jc], func=AF.Sqrt)
                for jj in range(jlo, jlo + nj):
                    gx = DWO[jj]
                    jjc = slice(jj, jj + 1)
                    nc.vector.tensor_mul(s2t[:, jjc], vcol("g2", i, gx), ve2[:, jjc])
                    nc.vector.tensor_mul(msq[:, jjc], mean2[:, jjc], s2t[:, jjc])
                    nc.vector.tensor_sub(msq[:, jjc], vcol("be2", i, gx), msq[:, jjc])
                    nc.vector.tensor_copy(out=t2bf[:, jjc], in_=msq[:, jjc])
                    nc.vector.tensor_scalar(
                        w2sc[:, gx * 128 : (gx + 1) * 128],
                        w2s[:, (i * G + gx) * 128 : (i * G + gx + 1) * 128],
                        s2t[:, jjc], None, ALU.mult,
                    )

            affine2_pair(0)
            affine2_pair(1)
            mvp = psum.tile([128, STW], F32, tag="big")
            for jj in range(G):
                gx = DWO[jj]
                nc.tensor.matmul(
                    mvp[:, 0:1],
                    w2s[:, (i * G + gx) * 128 : (i * G + gx + 1) * 128],
                    t2bf[:, jj : jj + 1],
                    start=(jj == 0), stop=(jj == G - 1),
                )
            nc.vector.tensor_scalar(
                b2p[:], mvp[:, 0:1], b2_s[:, i : i + 1], None, ALU.add)

            # ---- conv2 (D->C); pair-0 groups accumulate first so their
            # fills overlap the pair-1 gather. The conv2 bias is NOT applied
            # at drain time (layers 0-2): it folds into the next layer's
            # PReLU1 bias (b1x = b1' + W1' @ b2p), keeping the drains off
            # the collective critical path.
            for st, (s0, s1c) in enumerate(ST_COLS):
                ps = psum.tile([128, STW], F32, tag="big")
                for jj in range(G):
                    gx = DWO[jj]
                    for n0 in range(s0, s1c, NTW):
                        n1 = min(n0 + NTW, s1c)
                        nc.tensor.matmul(
                            ps[:, n0 - s0 : n1 - s0],
                            w2sc[:, gx * 128 : (gx + 1) * 128],
                            A[gx][:, PAD + n0 : PAD + n1],
                            start=(jj == 0), stop=(jj == G - 1),
                        )
                if last:
                    ys = ysp.tile([128, STW], F32, tag="ys")
                    nc.vector.scalar_tensor_tensor(
                        out=ys[:, 0 : s1c - s0],
                        in0=ps[:, 0 : s1c - s0],
                        scalar=b2p[:, 0:1],
                        in1=xs[:, s0:s1c],
                        op0=ALU.add, op1=ALU.add,
                    )
                    nc.sync.dma_start(out=yout[:, s0:s1c], in_=ys[:, 0 : s1c - s0])
                else:
                    nc.scalar.activation(
                        out=hb[i % 2][:, s0:s1c], in_=ps[:, 0 : s1c - s0],
                        func=AF.Identity, bias=b2p[:, 0:1], scale=1.0,
                    )


    nc.finalize()
    return nc


_CACHE = {}


def _get_program(a1, a2):
    key = (tuple(np.asarray(a1, dtype=np.float64)), tuple(np.asarray(a2, dtype=np.float64)))
    if key not in _CACHE:
        _CACHE[key] = _build_program(np.asarray(a1), np.asarray(a2))
    return _CACHE[key]


def _pack_params(w1, b1, g1, be1, wd, bd, g2, be2, w2, b2):
    w1 = np.asarray(w1, np.float32)
    w2 = np.asarray(w2, np.float32)
    wd = np.asarray(wd, np.float32)

    w1t = np.concatenate([w1[i].T for i in range(L)], axis=1)  # [C, L*D]
    w2t = np.concatenate(
        [w2[i].T[g * 128 : (g + 1) * 128] for i in range(L) for g in range(G)],
        axis=1,
    )
    dblocks = []
    for i in range(L):
        for g in range(G):
            for k in range(K):
                dblocks.append(np.diag(wd[i, g * 128 : (g + 1) * 128, k]))
    diag = np.concatenate(dblocks, axis=1)

    def pack16(tbl):
        out = np.empty((128, L * G), np.float32)
        for i in range(L):
            for g in range(G):
                out[:, i * G + g] = tbl[i, g * 128 : (g + 1) * 128]
        return out

    sw = wd.sum(axis=2)
    swL = wd[:, :, 1] + wd[:, :, 2]
    swR = wd[:, :, 0] + wd[:, :, 1]
    tables = {
        "b1": pack16(np.asarray(b1, np.float32)),
        "g1": pack16(np.asarray(g1, np.float32)),
        "be1": pack16(np.asarray(be1, np.float32)),
        "bd": pack16(np.asarray(bd, np.float32)),
        "swI": pack16(sw),
        "swL": pack16(swL),
        "swR": pack16(swR),
        "g2": pack16(np.asarray(g2, np.float32)),
        "be2": pack16(np.asarray(be2, np.float32)),
    }
    vec = np.concatenate([tables[t] for t in VEC_TABLES], axis=1)
    b2dv = np.asarray(b2, np.float32).T.copy()
    bfc = lambda a: np.ascontiguousarray(a).astype(ml_dtypes.bfloat16)
    return {
        "w1t": bfc(w1t),
        "w2t": bfc(w2t),
        "diag": bfc(diag),
        "vec": np.ascontiguousarray(vec),
        "b2d": b2dv,
    }


def kernel(x, w1, b1, a1, g1, be1, wd, bd, a2, g2, be2, w2, b2, _trace=False):
    x = np.asarray(x, np.float32)
    nc = _get_program(a1, a2)
    params = _pack_params(w1, b1, g1, be1, wd, bd, g2, be2, w2, b2)
    in_maps = [{"xin": np.ascontiguousarray(x[c]), **params} for c in range(NCORES)]
    res = run_bass_kernel_spmd(nc, in_maps, list(range(NCORES)), trace=_trace)
    out = np.stack([res.results[c]["yout"] for c in range(NCORES)], axis=0)
    kernel._last_result = res
    return out.astype(np.float32)
